# revision 1
# baseline (speedup 1.0000x reference)
"""Causal self-attention (T=4096, C=2048, 16 heads) on 8 TRN2 NeuronCores.

Sharding: tensor-parallel over heads (2 heads/core) for QKV + attention,
then per-head AllToAlls redistribute the attention output to
token-parallel (512 tokens/core) for the output projection. No reduction
collective is needed: each core computes full output rows for its token
slice and the host concatenates.

All matmuls run bf16 (inputs converted to bf16 on the host, halving DMA
bytes; PSUM accumulation stays fp32). Scores are computed transposed
(keys on partitions, queries free) so softmax denominators come from a
ones-vector matmul, P@V needs no transposes, and causal masking is a
bf16 multiply with 4 precomputed diagonal mask tiles; upper-triangle
blocks are skipped entirely. Softmax normalization is deferred across
the AllToAll: each A2A shard carries 128 rows of unnormalized P@V plus a
129th row with the softmax denominators, and the receiving side divides
- this keeps the phase-2 tensor-engine stream free of serializing
reciprocal chains (which otherwise reset the PE clock-warmup state).
"""
import sys
import types

sys.path.insert(0, "/opt/trn_rl_repo")

import ml_dtypes
import numpy as np

from concourse import bacc, tile
import concourse.mybir as mybir
from concourse.bass_utils import run_bass_kernel_spmd

F32 = mybir.dt.float32
BF16 = mybir.dt.bfloat16
NP_BF16 = np.dtype(ml_dtypes.bfloat16)

T, C = 4096, 2048
H, D = 16, 128
W = 8                  # cores
HL = H // W            # heads per core (2)
CL = HL * D            # local attention-output columns (256)
KT = C // 128          # contraction tiles (16)
TC1 = 512              # phase-1 token chunk
NC1 = T // TC1         # 8
TC2 = 512              # phase-2/3 token chunk
NC2 = T // TC2         # 8
TL = T // W            # tokens per core for the projection (512)
SCALE = float(1.0 / np.sqrt(D))

TRACE = False          # test harness sets kernel.TRACE = True for profiling
LAST_RESULT = {}       # test harness reads exec_time_ns from here

_cache = {}


def _build():
    nc = bacc.Bacc("TRN2", target_bir_lowering=False, debug=False, num_devices=W)
    xT_d = nc.dram_tensor("xT", [C, T], BF16, kind="ExternalInput")
    wqkT_d = nc.dram_tensor("wqkT", [C, 2 * CL], BF16, kind="ExternalInput")
    wvT_d = nc.dram_tensor("wvT", [C, CL], BF16, kind="ExternalInput")
    wpT_d = nc.dram_tensor("wpT", [C, C], BF16, kind="ExternalInput")
    out_d = nc.dram_tensor("out", [TL, C], F32, kind="ExternalOutput")

    with tile.TileContext(nc) as tc:
        with tc.tile_pool(name="res", bufs=1) as res, \
             tc.tile_pool(name="dram", bufs=1, space="DRAM") as dram:
            # per-head A2A buffers (bf16): shard j = my token chunk j.
            # att buffers keep 128-row shards (alignment matters for A2A
            # bandwidth); softmax denominators ride separate tiny A2As.
            a2a_in = [dram.tile([W, 128, TC2], BF16, tag=f"a2a_in{h}",
                                name=f"a2a_in{h}") for h in range(HL)]
            a2a_out = [dram.tile([W, 128, TC2], BF16, tag=f"a2a_out{h}",
                                 name=f"a2a_out{h}") for h in range(HL)]
            d2a_in = [dram.tile([W, 1, TC2], BF16, tag=f"d2a_in{h}",
                                name=f"d2a_in{h}") for h in range(HL)]
            d2a_out = [dram.tile([W, 1, TC2], BF16, tag=f"d2a_out{h}",
                                 name=f"d2a_out{h}") for h in range(HL)]

            # resident q/k (transposed, [d, t]) and V ([s, d]), all bf16
            qT = [res.tile([128, T], BF16, tag=f"qT{h}", name=f"qT{h}")
                  for h in range(HL)]
            kT = [res.tile([128, T], BF16, tag=f"kT{h}", name=f"kT{h}")
                  for h in range(HL)]
            V = [res.tile([128, CL], BF16, tag=f"V{i}", name=f"V{i}")
                 for i in range(T // 128)]

            ones32 = res.tile([128, 1], F32, tag="ones32")
            nc.gpsimd.memset(ones32[:], 1.0)
            ones = res.tile([128, 1], BF16, tag="ones")
            nc.vector.tensor_copy(ones[:], ones32[:])

            # 4 diagonal causal masks (keep where t >= s within the tile):
            # mask dk applies to s-tile k = 4j + dk of query chunk j
            masks = []
            for dk in range(4):
                m32 = res.tile([128, TC2], F32, tag=f"m32_{dk}",
                               name=f"m32_{dk}")
                nc.gpsimd.memset(m32[:], 1.0)
                mb = res.tile([128, TC2], BF16, tag=f"mask{dk}",
                              name=f"mask{dk}")
                nc.vector.tensor_copy(mb[:], m32[:])
                nc.gpsimd.affine_select(
                    out=mb[:], in_=mb[:],
                    compare_op=mybir.AluOpType.is_ge,
                    fill=0.0,
                    base=-128 * dk,
                    channel_multiplier=-1,
                    pattern=[[1, TC2]],
                )
                masks.append(mb)

            # ---------------- phase 1: QKV projection (bf16) ----------------
            with tc.tile_pool(name="wpool", bufs=1) as wpool, \
                 tc.tile_pool(name="xpool", bufs=2) as xpool, \
                 tc.tile_pool(name="ps1", bufs=3, space="PSUM") as ps1:
                wqk = [[None] * 4 for _ in range(KT)]

                def load_wqk(k):
                    for m in range(4):
                        t_ = wpool.tile([128, 128], BF16,
                                        tag=f"wqk{k}_{m}", name=f"wqk{k}_{m}")
                        nc.sync.dma_start(
                            t_[:],
                            wqkT_d.ap()[k * 128:(k + 1) * 128,
                                        m * 128:(m + 1) * 128],
                        )
                        wqk[k][m] = t_

                for k in range(KT):
                    load_wqk(k)

                def load_x_chunk(j):
                    xt = []
                    for k in range(KT):
                        t_ = xpool.tile([128, TC1], BF16, tag=f"x{k}",
                                        name=f"x{j}_{k}")
                        nc.sync.dma_start(
                            t_[:],
                            xT_d.ap()[k * 128:(k + 1) * 128,
                                      j * TC1:(j + 1) * TC1],
                        )
                        xt.append(t_)
                    return xt

                xt0 = load_x_chunk(0)
                for k in range(1, KT):
                    load_wqk(k)
                wv = []
                for k in range(KT):
                    t_ = wpool.tile([128, CL], BF16, tag=f"wv{k}", name=f"wv{k}")
                    nc.sync.dma_start(
                        t_[:], wvT_d.ap()[k * 128:(k + 1) * 128, :])
                    wv.append(t_)

                for j in range(NC1):
                    xt = xt0 if j == 0 else load_x_chunk(j)
                    # qT/kT for both heads: out[d, t] accumulated over c
                    for m in range(4):
                        pq = ps1.tile([128, TC1], F32, tag="pqk")
                        for k in range(KT):
                            nc.tensor.matmul(pq[:], wqk[k][m][:], xt[k][:],
                                             start=(k == 0), stop=(k == KT - 1))
                        dest = qT[m] if m < HL else kT[m - HL]
                        nc.vector.tensor_copy(
                            dest[:, j * TC1:(j + 1) * TC1], pq[:])
                    # V: out[t, d] accumulated over c
                    for tt in range(TC1 // 128):
                        pv = ps1.tile([128, CL], F32, tag="pv")
                        for k in range(KT):
                            nc.tensor.matmul(
                                pv[:],
                                xt[k][:, tt * 128:(tt + 1) * 128],
                                wv[k][:],
                                start=(k == 0), stop=(k == KT - 1))
                        nc.scalar.copy(V[j * (TC1 // 128) + tt][:], pv[:])

            # ---------------- phases 2+3 pools ----------------
            with tc.tile_pool(name="ph2", bufs=6) as p2, \
                 tc.tile_pool(name="a2s", bufs=3) as a2s, \
                 tc.tile_pool(name="p3a", bufs=1) as p3a, \
                 tc.tile_pool(name="p3n", bufs=2) as p3n, \
                 tc.tile_pool(name="p3w", bufs=1) as p3w, \
                 tc.tile_pool(name="p3o", bufs=2) as p3o:
                # prefetch the full projection weight during phase 2:
                # these DMAs sit on the sync queue ahead of the att writes
                wp = []
                for oc in range(C // 512):
                    row = []
                    for kc in range(KT):
                        t_ = p3w.tile([128, 512], BF16, tag=f"wp{oc}_{kc}",
                                      name=f"wp{oc}_{kc}")
                        nc.sync.dma_start(
                            t_[:],
                            wpT_d.ap()[kc * 128:(kc + 1) * 128,
                                       oc * 512:(oc + 1) * 512],
                        )
                        row.append(t_)
                    wp.append(row)

                # ---------------- phase 2: attention (bf16) ----------------
                with tc.tile_pool(name="ps2s", bufs=3, space="PSUM") as ps2s, \
                     tc.tile_pool(name="ps2o", bufs=2, space="PSUM") as ps2o, \
                     tc.tile_pool(name="ps2d", bufs=1, space="PSUM") as ps2d:
                    for h in range(HL):
                        for j in range(NC2):
                            nk = (j + 1) * (TC2 // 128)  # causal s tiles
                            po = ps2o.tile([128, TC2], F32, tag="po")
                            pd = ps2d.tile([1, TC2], F32, tag="pd")
                            for k in range(nk):
                                ps = ps2s.tile([128, TC2], F32, tag="ps")
                                nc.tensor.matmul(
                                    ps[:],
                                    kT[h][:, k * 128:(k + 1) * 128],
                                    qT[h][:, j * TC2:(j + 1) * TC2],
                                    start=True, stop=True)
                                e = p2.tile([128, TC2], BF16, tag="e")
                                nc.scalar.activation(
                                    e[:], ps[:],
                                    mybir.ActivationFunctionType.Exp,
                                    scale=SCALE)
                                dk = k - 4 * j
                                if dk >= 0:
                                    # diagonal tile: zero out s > t entries
                                    nc.vector.tensor_mul(e[:], e[:],
                                                         masks[dk][:])
                                nc.tensor.matmul(pd[:], ones[:], e[:],
                                                 start=(k == 0),
                                                 stop=(k == nk - 1))
                                nc.tensor.matmul(
                                    po[:],
                                    V[k][:, h * 128:(h + 1) * 128],
                                    e[:],
                                    start=(k == 0), stop=(k == nk - 1))
                            att = a2s.tile([128, TC2], BF16, tag="att")
                            nc.scalar.copy(att[:], po[:])
                            den = a2s.tile([1, TC2], BF16, tag="den")
                            nc.scalar.copy(den[:], pd[:])
                            nc.sync.dma_start(a2a_in[h][j, :, :], att[:])
                            nc.sync.dma_start(d2a_in[h][j, 0, :], den[:])
                        # fire this head's A2As as soon as its chunks are
                        # written; head 0's collectives overlap head 1
                        nc.gpsimd.collective_compute(
                            "AllToAll",
                            mybir.AluOpType.bypass,
                            ins=[a2a_in[h].opt()],
                            outs=[a2a_out[h].opt()],
                            replica_groups=[list(range(W))],
                        )
                        nc.gpsimd.collective_compute(
                            "AllToAll",
                            mybir.AluOpType.bypass,
                            ins=[d2a_in[h].opt()],
                            outs=[d2a_out[h].opt()],
                            replica_groups=[list(range(W))],
                        )

                # ---------------- phase 3: output projection (bf16) ----------------
                # load unnormalized attention + denominators, divide locally.
                # Per-head denominators arrive with that head's tiny A2A, so
                # head 0's normalization overlaps head 1's attention compute.
                attn = [None] * KT
                for h in range(HL):
                    den8 = p3n.tile([W, TL], BF16, tag=f"den8_{h}",
                                    name=f"den8_{h}")
                    for i in range(W):
                        kc = i * HL + h
                        t_ = p3a.tile([128, TL], BF16, tag=f"at{kc}",
                                      name=f"at{kc}")
                        nc.sync.dma_start(t_[:], a2a_out[h][i, :, :])
                        nc.sync.dma_start(den8[i:i + 1, :],
                                          d2a_out[h][i, 0, :])
                        attn[kc] = t_
                    den32 = p3n.tile([W, TL], F32, tag=f"den32_{h}",
                                     name=f"den32_{h}")
                    nc.vector.tensor_copy(den32[:], den8[:])
                    rec32 = p3n.tile([W, TL], F32, tag=f"rec32_{h}",
                                     name=f"rec32_{h}")
                    nc.vector.reciprocal(rec32[:], den32[:])
                    for i in range(W):
                        kc = i * HL + h
                        # row i -> partition 0 (DMA), then broadcast to 128;
                        # muls on gpsimd to keep DVE free for phase-2 masks
                        r1 = p3n.tile([1, TL], F32, tag="r1", bufs=4,
                                      name=f"r1_{kc}")
                        nc.sync.dma_start(r1[:], rec32[i:i + 1, :])
                        r128 = p3n.tile([128, TL], F32, tag="r128", bufs=3,
                                        name=f"r128_{kc}")
                        nc.gpsimd.partition_broadcast(r128[:], r1[:])
                        nc.gpsimd.tensor_mul(attn[kc][:], attn[kc][:], r128[:])
                # split accumulation: even kc (head-0 sourced, available
                # before the second A2A) first, so the tensor engine works
                # through the A2A/normalize latency of the odd tiles
                with tc.tile_pool(name="ps3", bufs=1, space="PSUM") as ps3:
                    for og in range(2):
                        po3s = {}
                        for oc in (2 * og, 2 * og + 1):
                            for tt in range(TL // 128):
                                po3 = ps3.tile([128, 512], F32,
                                               tag=f"po3_{oc % 2}_{tt}",
                                               name=f"po3_{oc}_{tt}")
                                po3s[(oc, tt)] = po3
                                for kc in range(0, KT, 2):
                                    nc.tensor.matmul(
                                        po3[:],
                                        attn[kc][:, tt * 128:(tt + 1) * 128],
                                        wp[oc][kc][:],
                                        start=(kc == 0), stop=False)
                        for oc in (2 * og, 2 * og + 1):
                            for tt in range(TL // 128):
                                po3 = po3s[(oc, tt)]
                                for kc in range(1, KT, 2):
                                    nc.tensor.matmul(
                                        po3[:],
                                        attn[kc][:, tt * 128:(tt + 1) * 128],
                                        wp[oc][kc][:],
                                        start=False, stop=(kc == KT - 1))
                                ob = p3o.tile([128, 512], F32, tag="ob")
                                nc.scalar.copy(ob[:], po3[:])
                                nc.sync.dma_start(
                                    out_d.ap()[tt * 128:(tt + 1) * 128,
                                               oc * 512:(oc + 1) * 512],
                                    ob[:])

    nc.compile()
    return nc


def _maybe_install_trace_hook():
    try:
        import antenv
        from trn_agent_boot.trn_boot import _ntff_profile_via_ctypes
        hook = _ntff_profile_via_ctypes("/opt/axon/libaxon_pjrt.so")
        mod = types.ModuleType("antenv.axon_hooks")
        mod.get_axon_ntff_profile_hook = lambda: hook
        mod.set_axon_ntff_profile_hook = lambda h: None
        sys.modules["antenv.axon_hooks"] = mod
        antenv.axon_hooks = mod
        return True
    except Exception:
        return False


def kernel(x, w_attn, w_proj):
    x = np.ascontiguousarray(x, dtype=np.float32)
    w_attn = np.ascontiguousarray(w_attn, dtype=np.float32)
    w_proj = np.ascontiguousarray(w_proj, dtype=np.float32)

    if "nc" not in _cache:
        _cache["nc"] = _build()
    nc = _cache["nc"]

    xT = np.ascontiguousarray(x.T).astype(NP_BF16)
    wpT = np.ascontiguousarray(w_proj.T).astype(NP_BF16)
    in_maps = []
    for c in range(W):
        r0 = CL * c
        wqk = np.concatenate(
            [w_attn[r0:r0 + CL], w_attn[C + r0:C + r0 + CL]], axis=0)
        wqkT = np.ascontiguousarray(wqk.T).astype(NP_BF16)
        wvT = np.ascontiguousarray(
            w_attn[2 * C + r0:2 * C + r0 + CL].T).astype(NP_BF16)
        in_maps.append({"xT": xT, "wqkT": wqkT, "wvT": wvT, "wpT": wpT})

    trace = TRACE and _maybe_install_trace_hook()
    res = run_bass_kernel_spmd(nc, in_maps, list(range(W)), trace=trace)
    LAST_RESULT["exec_time_ns"] = res.exec_time_ns

    return np.concatenate([res.results[c]["out"] for c in range(W)], axis=0)



# revision 2
# speedup vs baseline: 1.2512x; 1.2512x over previous
"""Causal self-attention (T=4096, C=2048, 16 heads) on 8 TRN2 NeuronCores.

Sharding: tensor-parallel over heads (2 heads/core) for QKV + attention,
then per-head AllToAlls redistribute the attention output to
token-parallel (512 tokens/core) for the output projection. No reduction
collective is needed: each core computes full output rows for its token
slice and the host concatenates.

All matmuls run 16-bit (bf16 weights/activations, fp16 on the
exp/V path; PSUM accumulation stays fp32). Scores are computed
transposed (keys on partitions, queries free). Softmax denominators are
accumulated OFF the tensor engine: exp tiles are summed into an fp16
esum on the vector engine (2x 16-bit mode), reduced across partitions
with one gpsimd partition_all_reduce per chunk (result arrives
broadcast on all 128 partitions), reciprocal'd on DVE, and the P@V
accumulator is normalized during PSUM evacuation. The AllToAll
therefore carries fully normalized attention output and phase 3 is a
pure DMA + matmul stream. Upper-triangle blocks are skipped entirely;
diagonal blocks are masked with 4 precomputed fp16 tiles.
"""
import sys
import types

sys.path.insert(0, "/opt/trn_rl_repo")

import ml_dtypes
import numpy as np

from concourse import bacc, tile
import concourse.mybir as mybir
import concourse.bass_isa as bass_isa
from concourse.bass_utils import run_bass_kernel_spmd

F32 = mybir.dt.float32
BF16 = mybir.dt.bfloat16
FP16 = mybir.dt.float16
NP_BF16 = np.dtype(ml_dtypes.bfloat16)

T, C = 4096, 2048
H, D = 16, 128
W = 8                  # cores
HL = H // W            # heads per core (2)
CL = HL * D            # local attention-output columns (256)
KT = C // 128          # contraction tiles (16)
TC1 = 512              # phase-1 token chunk
NC1 = T // TC1         # 8
TC2 = 512              # phase-2/3 token chunk
NC2 = T // TC2         # 8
TL = T // W            # tokens per core for the projection (512)
SCALE = float(1.0 / np.sqrt(D))

TRACE = False          # test harness sets kernel.TRACE = True for profiling
LAST_RESULT = {}       # test harness reads exec_time_ns from here

_cache = {}


def _build():
    nc = bacc.Bacc("TRN2", target_bir_lowering=False, debug=False, num_devices=W)
    xT_d = nc.dram_tensor("xT", [C, T], BF16, kind="ExternalInput")
    wqkT_d = nc.dram_tensor("wqkT", [C, 2 * CL], BF16, kind="ExternalInput")
    wvT_d = nc.dram_tensor("wvT", [C, CL], BF16, kind="ExternalInput")
    wpT_d = nc.dram_tensor("wpT", [C, C], BF16, kind="ExternalInput")
    out_d = nc.dram_tensor("out", [TL, C], F32, kind="ExternalOutput")

    with tile.TileContext(nc) as tc:
        with tc.tile_pool(name="res", bufs=1) as res, \
             tc.tile_pool(name="dram", bufs=1, space="DRAM") as dram:
            # per-head A2A buffers (bf16, normalized): shard j = token chunk j
            a2a_in = [dram.tile([W, 128, TC2], BF16, tag=f"a2a_in{h}",
                                name=f"a2a_in{h}") for h in range(HL)]
            a2a_out = [dram.tile([W, 128, TC2], BF16, tag=f"a2a_out{h}",
                                 name=f"a2a_out{h}") for h in range(HL)]

            # resident q/k (transposed, [d, t]) bf16 and V ([s, d]) fp16
            qT = [res.tile([128, T], BF16, tag=f"qT{h}", name=f"qT{h}")
                  for h in range(HL)]
            kT = [res.tile([128, T], BF16, tag=f"kT{h}", name=f"kT{h}")
                  for h in range(HL)]
            V = [res.tile([128, CL], FP16, tag=f"V{i}", name=f"V{i}")
                 for i in range(T // 128)]

            # 4 diagonal causal masks (keep where t >= s within the tile):
            # mask dk applies to s-tile k = 4j + dk of query chunk j
            masks = []
            for dk in range(4):
                m32 = res.tile([128, TC2], F32, tag=f"m32_{dk}",
                               name=f"m32_{dk}")
                nc.gpsimd.memset(m32[:], 1.0)
                mb = res.tile([128, TC2], FP16, tag=f"mask{dk}",
                              name=f"mask{dk}")
                nc.vector.tensor_copy(mb[:], m32[:])
                nc.gpsimd.affine_select(
                    out=mb[:], in_=mb[:],
                    compare_op=mybir.AluOpType.is_ge,
                    fill=0.0,
                    base=-128 * dk,
                    channel_multiplier=-1,
                    pattern=[[1, TC2]],
                )
                masks.append(mb)

            # ---------------- phase 1: QKV projection ----------------
            with tc.tile_pool(name="wpool", bufs=1) as wpool, \
                 tc.tile_pool(name="xpool", bufs=2) as xpool, \
                 tc.tile_pool(name="ps1", bufs=3, space="PSUM") as ps1:
                wqk = [[None] * 4 for _ in range(KT)]

                def load_wqk(k):
                    for m in range(4):
                        t_ = wpool.tile([128, 128], BF16,
                                        tag=f"wqk{k}_{m}", name=f"wqk{k}_{m}")
                        nc.sync.dma_start(
                            t_[:],
                            wqkT_d.ap()[k * 128:(k + 1) * 128,
                                        m * 128:(m + 1) * 128],
                        )
                        wqk[k][m] = t_

                def load_x_chunk(j):
                    xt = []
                    for k in range(KT):
                        t_ = xpool.tile([128, TC1], BF16, tag=f"x{k}",
                                        name=f"x{j}_{k}")
                        nc.sync.dma_start(
                            t_[:],
                            xT_d.ap()[k * 128:(k + 1) * 128,
                                      j * TC1:(j + 1) * TC1],
                        )
                        xt.append(t_)
                    return xt

                load_wqk(0)
                xt0 = load_x_chunk(0)
                for k in range(1, KT):
                    load_wqk(k)
                wv = []
                for k in range(KT):
                    t_ = wpool.tile([128, CL], BF16, tag=f"wv{k}", name=f"wv{k}")
                    nc.sync.dma_start(
                        t_[:], wvT_d.ap()[k * 128:(k + 1) * 128, :])
                    wv.append(t_)

                for j in range(NC1):
                    xt = xt0 if j == 0 else load_x_chunk(j)
                    # qT/kT for both heads: out[d, t] accumulated over c
                    for m in range(4):
                        pq = ps1.tile([128, TC1], F32, tag="pqk")
                        for k in range(KT):
                            nc.tensor.matmul(pq[:], wqk[k][m][:], xt[k][:],
                                             start=(k == 0), stop=(k == KT - 1))
                        dest = qT[m] if m < HL else kT[m - HL]
                        nc.vector.tensor_copy(
                            dest[:, j * TC1:(j + 1) * TC1], pq[:])
                    # V: out[t, d] accumulated over c
                    for tt in range(TC1 // 128):
                        pv = ps1.tile([128, CL], F32, tag="pv")
                        for k in range(KT):
                            nc.tensor.matmul(
                                pv[:],
                                xt[k][:, tt * 128:(tt + 1) * 128],
                                wv[k][:],
                                start=(k == 0), stop=(k == KT - 1))
                        nc.scalar.copy(V[j * (TC1 // 128) + tt][:], pv[:])

            # ---------------- phases 2+3 pools ----------------
            with tc.tile_pool(name="ph2", bufs=6) as p2, \
                 tc.tile_pool(name="es", bufs=2) as es, \
                 tc.tile_pool(name="dn", bufs=2) as dn, \
                 tc.tile_pool(name="a2s", bufs=3) as a2s, \
                 tc.tile_pool(name="p3a", bufs=1) as p3a, \
                 tc.tile_pool(name="p3w", bufs=1) as p3w, \
                 tc.tile_pool(name="p3o", bufs=2) as p3o:
                # prefetch the full projection weight during phase 2:
                # these DMAs sit on the sync queue ahead of the att writes
                wp = []
                for oc in range(C // 512):
                    row = []
                    for kc in range(KT):
                        t_ = p3w.tile([128, 512], BF16, tag=f"wp{oc}_{kc}",
                                      name=f"wp{oc}_{kc}")
                        nc.sync.dma_start(
                            t_[:],
                            wpT_d.ap()[kc * 128:(kc + 1) * 128,
                                       oc * 512:(oc + 1) * 512],
                        )
                        row.append(t_)
                    wp.append(row)

                attn = [None] * KT

                def load_attn(h):
                    for i in range(W):
                        kc = i * HL + h
                        t_ = p3a.tile([128, TL], BF16, tag=f"at{kc}",
                                      name=f"at{kc}")
                        nc.sync.dma_start(t_[:], a2a_out[h][i, :, :])
                        attn[kc] = t_

                # ---------------- phase 2: attention ----------------
                with tc.tile_pool(name="ps2s", bufs=4, space="PSUM") as ps2s, \
                     tc.tile_pool(name="ps2o", bufs=2, space="PSUM") as ps2o:
                    for h in range(HL):
                        for j in range(NC2):
                            nk = (j + 1) * (TC2 // 128)  # causal s tiles
                            po = ps2o.tile([128, TC2], F32, tag="po")
                            qs = qT[h][:, j * TC2:(j + 1) * TC2]
                            pss = {}

                            def emit_score(k):
                                ps = ps2s.tile([128, TC2], F32, tag="ps")
                                nc.tensor.matmul(
                                    ps[:],
                                    kT[h][:, k * 128:(k + 1) * 128],
                                    qs, start=True, stop=True)
                                pss[k] = ps

                            for k in range(min(3, nk)):
                                emit_score(k)
                            esum = es.tile([128, TC2], FP16, tag="esum")
                            for k in range(nk):
                                ps = pss.pop(k)
                                e = p2.tile([128, TC2], FP16, tag="e")
                                nc.scalar.activation(
                                    e[:], ps[:],
                                    mybir.ActivationFunctionType.Exp,
                                    scale=SCALE)
                                dk = k - 4 * j
                                if dk >= 0:
                                    # diagonal tile: zero out s > t entries
                                    nc.vector.tensor_mul(e[:], e[:],
                                                         masks[dk][:])
                                if k == 0:
                                    nc.vector.tensor_copy(esum[:], e[:])
                                else:
                                    nc.vector.tensor_add(esum[:], esum[:],
                                                         e[:])
                                if k + 3 < nk:
                                    emit_score(k + 3)
                                nc.tensor.matmul(
                                    po[:],
                                    V[k][:, h * 128:(h + 1) * 128],
                                    e[:],
                                    start=(k == 0), stop=(k == nk - 1))
                            # denominators: partition-reduce esum (result is
                            # broadcast to all 128 partitions), reciprocal,
                            # then normalize po during PSUM evacuation
                            den = dn.tile([128, TC2], F32, tag="den")
                            nc.gpsimd.partition_all_reduce(
                                den[:], esum[:], channels=128,
                                reduce_op=bass_isa.ReduceOp.add)
                            rec = dn.tile([128, TC2], F32, tag="rec")
                            nc.vector.reciprocal_approx_fast(
                                out=rec[:], in_=den[:])
                            att = a2s.tile([128, TC2], BF16, tag="att")
                            nc.vector.tensor_mul(att[:], po[:], rec[:])
                            nc.sync.dma_start(a2a_in[h][j, :, :], att[:])
                        # fire this head's A2A as soon as its chunks are
                        # written; head 0's collective overlaps head 1
                        nc.gpsimd.collective_compute(
                            "AllToAll",
                            mybir.AluOpType.bypass,
                            ins=[a2a_in[h].opt()],
                            outs=[a2a_out[h].opt()],
                            replica_groups=[list(range(W))],
                        )
                        load_attn(h)

                # ---------------- phase 3: output projection ----------------
                # attn tiles arrive normalized; even kc (head-0 sourced,
                # available before the second A2A) first, so the tensor
                # engine works through the A2A latency of the odd tiles
                with tc.tile_pool(name="ps3", bufs=1, space="PSUM") as ps3:
                    for og in range(2):
                        ocs = (2 * og, 2 * og + 1)
                        po3s = {}
                        for oc in ocs:
                            for tt in range(TL // 128):
                                po3s[(oc, tt)] = ps3.tile(
                                    [128, 512], F32,
                                    tag=f"po3_{oc % 2}_{tt}",
                                    name=f"po3_{oc}_{tt}")
                        for par in range(2):
                            for tt in range(TL // 128):
                                for kc in range(par, KT, 2):
                                    lhs = attn[kc][:, tt * 128:(tt + 1) * 128]
                                    for oc in ocs:
                                        nc.tensor.matmul(
                                            po3s[(oc, tt)][:],
                                            lhs,
                                            wp[oc][kc][:],
                                            start=(kc == 0),
                                            stop=(kc == KT - 1))
                        for oc in ocs:
                            for tt in range(TL // 128):
                                ob = p3o.tile([128, 512], F32, tag="ob")
                                nc.scalar.copy(ob[:], po3s[(oc, tt)][:])
                                nc.sync.dma_start(
                                    out_d.ap()[tt * 128:(tt + 1) * 128,
                                               oc * 512:(oc + 1) * 512],
                                    ob[:])

    nc.compile()
    return nc


def _maybe_install_trace_hook():
    try:
        import antenv
        from trn_agent_boot.trn_boot import _ntff_profile_via_ctypes
        hook = _ntff_profile_via_ctypes("/opt/axon/libaxon_pjrt.so")
        mod = types.ModuleType("antenv.axon_hooks")
        mod.get_axon_ntff_profile_hook = lambda: hook
        mod.set_axon_ntff_profile_hook = lambda h: None
        sys.modules["antenv.axon_hooks"] = mod
        antenv.axon_hooks = mod
        return True
    except Exception:
        return False


def kernel(x, w_attn, w_proj):
    x = np.ascontiguousarray(x, dtype=np.float32)
    w_attn = np.ascontiguousarray(w_attn, dtype=np.float32)
    w_proj = np.ascontiguousarray(w_proj, dtype=np.float32)

    if "nc" not in _cache:
        _cache["nc"] = _build()
    nc = _cache["nc"]

    xT = np.ascontiguousarray(x.T).astype(NP_BF16)
    wpT = np.ascontiguousarray(w_proj.T).astype(NP_BF16)
    in_maps = []
    for c in range(W):
        r0 = CL * c
        wqk = np.concatenate(
            [w_attn[r0:r0 + CL], w_attn[C + r0:C + r0 + CL]], axis=0)
        wqkT = np.ascontiguousarray(wqk.T).astype(NP_BF16)
        wvT = np.ascontiguousarray(
            w_attn[2 * C + r0:2 * C + r0 + CL].T).astype(NP_BF16)
        in_maps.append({"xT": xT, "wqkT": wqkT, "wvT": wvT, "wpT": wpT})

    trace = TRACE and _maybe_install_trace_hook()
    res = run_bass_kernel_spmd(nc, in_maps, list(range(W)), trace=trace)
    LAST_RESULT["exec_time_ns"] = res.exec_time_ns

    return np.concatenate([res.results[c]["out"] for c in range(W)], axis=0)


# revision 6
# speedup vs baseline: 1.3439x; 1.0741x over previous
"""Causal self-attention (T=4096, C=2048, 16 heads) on 8 TRN2 NeuronCores.

Sharding: tensor-parallel over heads (2 heads/core) for QKV + attention,
then per-head AllToAlls redistribute the attention output to
token-parallel (512 tokens/core) for the output projection. No reduction
collective is needed: each core computes full output rows for its token
slice and the host concatenates.

All matmuls run 16-bit (bf16 weights/activations, fp16 on the
exp/V path; PSUM accumulation stays fp32). Scores are computed
transposed (keys on partitions, queries free). Softmax denominators are
accumulated OFF the tensor engine: exp tiles are summed into an fp16
esum on the vector engine (2x 16-bit mode), reduced across partitions
with one gpsimd partition_all_reduce per chunk (result arrives
broadcast on all 128 partitions), reciprocal'd on DVE, and the P@V
accumulator is normalized during PSUM evacuation. The AllToAll
therefore carries fully normalized attention output and phase 3 is a
pure DMA + matmul stream. Upper-triangle blocks are skipped entirely;
diagonal blocks are masked with 4 precomputed fp16 tiles.
"""
import sys
import types

sys.path.insert(0, "/opt/trn_rl_repo")

import ml_dtypes
import numpy as np

from concourse import bacc, tile
import concourse.mybir as mybir
import concourse.bass_isa as bass_isa
from concourse.bass_utils import run_bass_kernel_spmd

F32 = mybir.dt.float32
BF16 = mybir.dt.bfloat16
FP16 = mybir.dt.float16
NP_BF16 = np.dtype(ml_dtypes.bfloat16)

T, C = 4096, 2048
H, D = 16, 128
W = 8                  # cores
HL = H // W            # heads per core (2)
CL = HL * D            # local attention-output columns (256)
KT = C // 128          # contraction tiles (16)
TC1 = 512              # phase-1 token chunk
NC1 = T // TC1         # 8
TC2 = 512              # phase-2/3 token chunk
NC2 = T // TC2         # 8
TL = T // W            # tokens per core for the projection (512)
SCALE = float(1.0 / np.sqrt(D))

TRACE = False          # test harness sets kernel.TRACE = True for profiling
LAST_RESULT = {}       # test harness reads exec_time_ns from here

_cache = {}


def _build():
    nc = bacc.Bacc("TRN2", target_bir_lowering=False, debug=False, num_devices=W)
    xT_d = nc.dram_tensor("xT", [C, T], BF16, kind="ExternalInput")
    wqkT_d = nc.dram_tensor("wqkT", [C, 2 * CL], BF16, kind="ExternalInput")
    wvT_d = nc.dram_tensor("wvT", [C, CL], BF16, kind="ExternalInput")
    wpT_d = nc.dram_tensor("wpT", [C, C], BF16, kind="ExternalInput")
    out_d = nc.dram_tensor("out", [TL, C], F32, kind="ExternalOutput")

    with tile.TileContext(nc) as tc:
        with tc.tile_pool(name="res", bufs=1) as res, \
             tc.tile_pool(name="dram", bufs=1, space="DRAM") as dram:
            # per-head A2A buffers (bf16, normalized): shard j = token chunk j
            a2a_in = [dram.tile([W, 128, TC2], BF16, tag=f"a2a_in{h}",
                                name=f"a2a_in{h}") for h in range(HL)]
            a2a_out = [dram.tile([W, 128, TC2], BF16, tag=f"a2a_out{h}",
                                 name=f"a2a_out{h}") for h in range(HL)]

            # resident q/k (transposed, [d, t]) and V ([s, d]), fp16
            qT = [res.tile([128, T], FP16, tag=f"qT{h}", name=f"qT{h}")
                  for h in range(HL)]
            kT = [res.tile([128, T], FP16, tag=f"kT{h}", name=f"kT{h}")
                  for h in range(HL)]
            V = [res.tile([128, CL], FP16, tag=f"V{i}", name=f"V{i}")
                 for i in range(T // 128)]

            # 4 diagonal causal masks (keep where t >= s within the tile):
            # mask dk applies to s-tile k = 4j + dk of query chunk j
            masks = []
            for dk in range(4):
                m32 = res.tile([128, TC2], F32, tag=f"m32_{dk}",
                               name=f"m32_{dk}")
                nc.gpsimd.memset(m32[:], 1.0)
                mb = res.tile([128, TC2], FP16, tag=f"mask{dk}",
                              name=f"mask{dk}")
                nc.vector.tensor_copy(mb[:], m32[:])
                nc.gpsimd.affine_select(
                    out=mb[:], in_=mb[:],
                    compare_op=mybir.AluOpType.is_ge,
                    fill=0.0,
                    base=-128 * dk,
                    channel_multiplier=-1,
                    pattern=[[1, TC2]],
                )
                masks.append(mb)

            # ---------------- phase 1: QKV projection ----------------
            with tc.tile_pool(name="wpool", bufs=1) as wpool, \
                 tc.tile_pool(name="xpool", bufs=2) as xpool, \
                 tc.tile_pool(name="ps1", bufs=4, space="PSUM") as ps1:
                wqk = [[None] * 4 for _ in range(KT)]

                def load_wqk(k):
                    for m in range(4):
                        t_ = wpool.tile([128, 128], BF16,
                                        tag=f"wqk{k}_{m}", name=f"wqk{k}_{m}")
                        nc.sync.dma_start(
                            t_[:],
                            wqkT_d.ap()[k * 128:(k + 1) * 128,
                                        m * 128:(m + 1) * 128],
                        )
                        wqk[k][m] = t_

                def load_x_chunk(j):
                    xt = []
                    for k in range(KT):
                        t_ = xpool.tile([128, TC1], BF16, tag=f"x{k}",
                                        name=f"x{j}_{k}")
                        nc.sync.dma_start(
                            t_[:],
                            xT_d.ap()[k * 128:(k + 1) * 128,
                                      j * TC1:(j + 1) * TC1],
                        )
                        xt.append(t_)
                    return xt

                load_wqk(0)
                xt0 = load_x_chunk(0)
                for k in range(1, KT):
                    load_wqk(k)
                wv = []
                for k in range(KT):
                    t_ = wpool.tile([128, CL], BF16, tag=f"wv{k}", name=f"wv{k}")
                    nc.sync.dma_start(
                        t_[:], wvT_d.ap()[k * 128:(k + 1) * 128, :])
                    wv.append(t_)

                for j in range(NC1):
                    xt = xt0 if j == 0 else load_x_chunk(j)
                    # qT/kT for both heads: out[d, t] accumulated over c
                    for m in range(4):
                        pq = ps1.tile([128, TC1], F32, tag="pqk")
                        for k in range(KT):
                            nc.tensor.matmul(pq[:], wqk[k][m][:], xt[k][:],
                                             start=(k == 0), stop=(k == KT - 1))
                        dest = qT[m] if m < HL else kT[m - HL]
                        nc.vector.tensor_copy(
                            dest[:, j * TC1:(j + 1) * TC1], pq[:])
                    # V: out[t, d] accumulated over c
                    for tt in range(TC1 // 128):
                        pv = ps1.tile([128, CL], F32, tag="pv")
                        for k in range(KT):
                            nc.tensor.matmul(
                                pv[:],
                                xt[k][:, tt * 128:(tt + 1) * 128],
                                wv[k][:],
                                start=(k == 0), stop=(k == KT - 1))
                        nc.scalar.copy(V[j * (TC1 // 128) + tt][:], pv[:])

            # ---------------- phases 2+3 pools ----------------
            with tc.tile_pool(name="ph2", bufs=4) as p2, \
                 tc.tile_pool(name="es", bufs=2) as es, \
                 tc.tile_pool(name="dn", bufs=2) as dn, \
                 tc.tile_pool(name="a2s", bufs=3) as a2s, \
                 tc.tile_pool(name="p3a", bufs=1) as p3a, \
                 tc.tile_pool(name="p3w", bufs=1) as p3w, \
                 tc.tile_pool(name="acc3", bufs=1) as acc3, \
                 tc.tile_pool(name="p3o", bufs=2) as p3o:
                # prefetch the full projection weight during phase 2:
                # these DMAs sit on the sync queue ahead of the att writes
                wp = []
                for oc in range(C // 512):
                    row = []
                    for kc in range(KT):
                        t_ = p3w.tile([128, 512], BF16, tag=f"wp{oc}_{kc}",
                                      name=f"wp{oc}_{kc}")
                        nc.sync.dma_start(
                            t_[:],
                            wpT_d.ap()[kc * 128:(kc + 1) * 128,
                                       oc * 512:(oc + 1) * 512],
                        )
                        row.append(t_)
                    wp.append(row)

                attn = [None] * KT

                def load_attn(h):
                    for i in range(W):
                        kc = i * HL + h
                        t_ = p3a.tile([128, TL], BF16, tag=f"at{kc}",
                                      name=f"at{kc}")
                        nc.sync.dma_start(t_[:], a2a_out[h][i, :, :])
                        attn[kc] = t_

                # one PSUM pool level across phases 2+3 (8 banks total):
                # paired scores 2x[128,1024] (4 banks) + po 2 + phase-3 2
                with tc.tile_pool(name="ps2s", bufs=2, space="PSUM") as ps2s, \
                     tc.tile_pool(name="ps2o", bufs=2, space="PSUM") as ps2o, \
                     tc.tile_pool(name="ps3", bufs=2, space="PSUM") as ps3:
                    # ------------- phase 2: attention -------------
                    for h in range(HL):
                        for j in range(NC2):
                            nk = (j + 1) * (TC2 // 128)  # causal s tiles
                            npair = nk // 2
                            po = ps2o.tile([128, TC2], F32, tag="po")
                            qs = qT[h][:, j * TC2:(j + 1) * TC2]
                            pss = {}

                            def emit_pair(p):
                                # two score blocks into one 2-bank psum tile
                                ps = ps2s.tile([128, 2 * TC2], F32, tag="ps")
                                for half in range(2):
                                    k = 2 * p + half
                                    nc.tensor.matmul(
                                        ps[:, half * TC2:(half + 1) * TC2],
                                        kT[h][:, k * 128:(k + 1) * 128],
                                        qs, start=True, stop=True)
                                pss[p] = ps

                            for p in range(min(2, npair)):
                                emit_pair(p)
                            esum = es.tile([128, TC2], FP16, tag="esum")
                            for p in range(npair):
                                ps = pss.pop(p)
                                e = p2.tile([128, 2 * TC2], FP16, tag="e")
                                # one exp per pair amortizes ACT init cost
                                nc.scalar.activation(
                                    e[:], ps[:],
                                    mybir.ActivationFunctionType.Exp,
                                    scale=SCALE)
                                for half in range(2):
                                    k = 2 * p + half
                                    eh = e[:, half * TC2:(half + 1) * TC2]
                                    dk = k - 4 * j
                                    if dk >= 0:
                                        # diagonal: zero out s > t entries
                                        nc.vector.tensor_mul(eh, eh,
                                                             masks[dk][:])
                                    if k == 0:
                                        nc.vector.tensor_copy(esum[:], eh)
                                    else:
                                        nc.vector.tensor_add(esum[:],
                                                             esum[:], eh)
                                if p + 2 < npair:
                                    emit_pair(p + 2)
                                for half in range(2):
                                    k = 2 * p + half
                                    nc.tensor.matmul(
                                        po[:],
                                        V[k][:, h * 128:(h + 1) * 128],
                                        e[:, half * TC2:(half + 1) * TC2],
                                        start=(k == 0), stop=(k == nk - 1))
                            # denominators: partition-reduce esum (result is
                            # broadcast to all 128 partitions), reciprocal,
                            # then normalize po during PSUM evacuation
                            den = dn.tile([128, TC2], F32, tag="den")
                            nc.gpsimd.partition_all_reduce(
                                den[:], esum[:], channels=128,
                                reduce_op=bass_isa.ReduceOp.add)
                            rec = dn.tile([128, TC2], F32, tag="rec")
                            nc.vector.reciprocal_approx_fast(
                                out=rec[:], in_=den[:])
                            att = a2s.tile([128, TC2], BF16, tag="att")
                            nc.vector.tensor_mul(att[:], po[:], rec[:])
                            nc.sync.dma_start(a2a_in[h][j, :, :], att[:])
                        # fire this head's A2A as soon as its chunks are
                        # written; head 0's collective overlaps head 1
                        nc.gpsimd.collective_compute(
                            "AllToAll",
                            mybir.AluOpType.bypass,
                            ins=[a2a_in[h].opt()],
                            outs=[a2a_out[h].opt()],
                            replica_groups=[list(range(W))],
                        )
                        load_attn(h)

                    # ------------- phase 3: output projection -------------
                    # attn arrives normalized. All 16 groups' even-kc halves
                    # (head-0 sourced, available before the second A2A) run
                    # first into rotating psum banks with f32 SBUF spill, so
                    # the tensor engine covers the A2A + odd-tile DMA window;
                    # odd halves then finish in psum and DVE adds the spill.
                    accs = {}
                    for oc in range(4):
                        for tt in range(TL // 128):
                            p3 = ps3.tile([128, 512], F32, tag="p3")
                            for kc in range(0, KT, 2):
                                nc.tensor.matmul(
                                    p3[:],
                                    attn[kc][:, tt * 128:(tt + 1) * 128],
                                    wp[oc][kc][:],
                                    start=(kc == 0), stop=(kc == KT - 2))
                            acc = acc3.tile([128, 512], F32,
                                            tag=f"acc{oc}_{tt}",
                                            name=f"acc{oc}_{tt}")
                            nc.scalar.copy(acc[:], p3[:])
                            accs[(oc, tt)] = acc
                    for oc in range(4):
                        for tt in range(TL // 128):
                            p3 = ps3.tile([128, 512], F32, tag="p3")
                            for kc in range(1, KT, 2):
                                nc.tensor.matmul(
                                    p3[:],
                                    attn[kc][:, tt * 128:(tt + 1) * 128],
                                    wp[oc][kc][:],
                                    start=(kc == 1), stop=(kc == KT - 1))
                            ob = p3o.tile([128, 512], F32, tag="ob")
                            nc.vector.tensor_add(ob[:], accs[(oc, tt)][:],
                                                 p3[:])
                            nc.sync.dma_start(
                                out_d.ap()[tt * 128:(tt + 1) * 128,
                                           oc * 512:(oc + 1) * 512],
                                ob[:])

    nc.compile()
    return nc


def _maybe_install_trace_hook():
    try:
        import antenv
        from trn_agent_boot.trn_boot import _ntff_profile_via_ctypes
        hook = _ntff_profile_via_ctypes("/opt/axon/libaxon_pjrt.so")
        mod = types.ModuleType("antenv.axon_hooks")
        mod.get_axon_ntff_profile_hook = lambda: hook
        mod.set_axon_ntff_profile_hook = lambda h: None
        sys.modules["antenv.axon_hooks"] = mod
        antenv.axon_hooks = mod
        return True
    except Exception:
        return False


def kernel(x, w_attn, w_proj):
    x = np.ascontiguousarray(x, dtype=np.float32)
    w_attn = np.ascontiguousarray(w_attn, dtype=np.float32)
    w_proj = np.ascontiguousarray(w_proj, dtype=np.float32)

    if "nc" not in _cache:
        _cache["nc"] = _build()
    nc = _cache["nc"]

    xT = np.ascontiguousarray(x.T).astype(NP_BF16)
    wpT = np.ascontiguousarray(w_proj.T).astype(NP_BF16)
    in_maps = []
    for c in range(W):
        r0 = CL * c
        wqk = np.concatenate(
            [w_attn[r0:r0 + CL], w_attn[C + r0:C + r0 + CL]], axis=0)
        wqkT = np.ascontiguousarray(wqk.T).astype(NP_BF16)
        wvT = np.ascontiguousarray(
            w_attn[2 * C + r0:2 * C + r0 + CL].T).astype(NP_BF16)
        in_maps.append({"xT": xT, "wqkT": wqkT, "wvT": wvT, "wpT": wpT})

    trace = TRACE and _maybe_install_trace_hook()
    res = run_bass_kernel_spmd(nc, in_maps, list(range(W)), trace=trace)
    LAST_RESULT["exec_time_ns"] = res.exec_time_ns

    return np.concatenate([res.results[c]["out"] for c in range(W)], axis=0)


# revision 10
# speedup vs baseline: 1.3635x; 1.0146x over previous
"""Causal self-attention (T=4096, C=2048, 16 heads) on 8 TRN2 NeuronCores.

Sharding: tensor-parallel over heads (2 heads/core) for QKV + attention,
then per-head AllToAlls redistribute the attention output to
token-parallel (512 tokens/core) for the output projection. No reduction
collective is needed: each core computes full output rows for its token
slice and the host concatenates.

All matmuls run 16-bit (bf16 weights/activations, fp16 on the
exp/V path; PSUM accumulation stays fp32). Scores are computed
transposed (keys on partitions, queries free). Softmax denominators are
accumulated OFF the tensor engine: exp tiles are summed into an fp16
esum on the vector engine (2x 16-bit mode), reduced across partitions
with one gpsimd partition_all_reduce per chunk (result arrives
broadcast on all 128 partitions), reciprocal'd on DVE, and the P@V
accumulator is normalized during PSUM evacuation. The AllToAll
therefore carries fully normalized attention output and phase 3 is a
pure DMA + matmul stream. Upper-triangle blocks are skipped entirely;
diagonal blocks are masked with 4 precomputed fp16 tiles.
"""
import sys
import types

sys.path.insert(0, "/opt/trn_rl_repo")

import ml_dtypes
import numpy as np

from concourse import bacc, tile
import concourse.mybir as mybir
import concourse.bass_isa as bass_isa
from concourse.bass_utils import run_bass_kernel_spmd

F32 = mybir.dt.float32
BF16 = mybir.dt.bfloat16
FP16 = mybir.dt.float16
NP_BF16 = np.dtype(ml_dtypes.bfloat16)

T, C = 4096, 2048
H, D = 16, 128
W = 8                  # cores
HL = H // W            # heads per core (2)
CL = HL * D            # local attention-output columns (256)
KT = C // 128          # contraction tiles (16)
TC1 = 512              # phase-1 token chunk
NC1 = T // TC1         # 8
TC2 = 512              # phase-2/3 token chunk
NC2 = T // TC2         # 8
TL = T // W            # tokens per core for the projection (512)
SCALE = float(1.0 / np.sqrt(D))

TRACE = False          # test harness sets kernel.TRACE = True for profiling
LAST_RESULT = {}       # test harness reads exec_time_ns from here

_cache = {}


def _build():
    nc = bacc.Bacc("TRN2", target_bir_lowering=False, debug=False, num_devices=W)
    xT_d = nc.dram_tensor("xT", [C, T], BF16, kind="ExternalInput")
    wqkT_d = nc.dram_tensor("wqkT", [C, 2 * CL], BF16, kind="ExternalInput")
    wvT_d = nc.dram_tensor("wvT", [C, CL], BF16, kind="ExternalInput")
    wpT_d = nc.dram_tensor("wpT", [C, C], BF16, kind="ExternalInput")
    out_d = nc.dram_tensor("out", [TL, C], F32, kind="ExternalOutput")

    with tile.TileContext(nc) as tc:
        with tc.tile_pool(name="res", bufs=1) as res, \
             tc.tile_pool(name="dram", bufs=1, space="DRAM") as dram:
            # per-head A2A buffers (bf16, normalized): shard j = token chunk j
            a2a_in = [dram.tile([W, 128, TC2], BF16, tag=f"a2a_in{h}",
                                name=f"a2a_in{h}") for h in range(HL)]
            a2a_out = [dram.tile([W, 128, TC2], BF16, tag=f"a2a_out{h}",
                                 name=f"a2a_out{h}") for h in range(HL)]

            # resident q/k (transposed, [d, t]) and V ([s, d]), fp16
            qT = [res.tile([128, T], FP16, tag=f"qT{h}", name=f"qT{h}")
                  for h in range(HL)]
            kT = [res.tile([128, T], FP16, tag=f"kT{h}", name=f"kT{h}")
                  for h in range(HL)]
            V = [res.tile([128, CL], FP16, tag=f"V{i}", name=f"V{i}")
                 for i in range(T // 128)]

            # 4 diagonal causal masks (keep where t >= s within the tile):
            # mask dk applies to s-tile k = 4j + dk of query chunk j
            masks = []
            for dk in range(4):
                m32 = res.tile([128, TC2], F32, tag=f"m32_{dk}",
                               name=f"m32_{dk}")
                nc.gpsimd.memset(m32[:], 1.0)
                mb = res.tile([128, TC2], FP16, tag=f"mask{dk}",
                              name=f"mask{dk}")
                nc.vector.tensor_copy(mb[:], m32[:])
                nc.gpsimd.affine_select(
                    out=mb[:], in_=mb[:],
                    compare_op=mybir.AluOpType.is_ge,
                    fill=0.0,
                    base=-128 * dk,
                    channel_multiplier=-1,
                    pattern=[[1, TC2]],
                )
                masks.append(mb)

            # ---------------- phase 1: QKV projection ----------------
            with tc.tile_pool(name="wpool", bufs=1) as wpool, \
                 tc.tile_pool(name="xpool", bufs=2) as xpool, \
                 tc.tile_pool(name="ps1", bufs=4, space="PSUM") as ps1:
                wqk = [[None] * 4 for _ in range(KT)]

                def load_wqk(k):
                    for m in range(4):
                        t_ = wpool.tile([128, 128], BF16,
                                        tag=f"wqk{k}_{m}", name=f"wqk{k}_{m}")
                        nc.sync.dma_start(
                            t_[:],
                            wqkT_d.ap()[k * 128:(k + 1) * 128,
                                        m * 128:(m + 1) * 128],
                        )
                        wqk[k][m] = t_

                def load_x_chunk(j):
                    xt = []
                    for k in range(KT):
                        t_ = xpool.tile([128, TC1], BF16, tag=f"x{k}",
                                        name=f"x{j}_{k}")
                        nc.sync.dma_start(
                            t_[:],
                            xT_d.ap()[k * 128:(k + 1) * 128,
                                      j * TC1:(j + 1) * TC1],
                        )
                        xt.append(t_)
                    return xt

                # interleave chunk-0 x tiles, wqk, and wv per k so the first
                # chunk's matmuls can start as soon as ~300KB has landed
                wv = []
                xt0 = []
                for k in range(KT):
                    t_ = xpool.tile([128, TC1], BF16, tag=f"x{k}",
                                    name=f"x0_{k}")
                    nc.sync.dma_start(
                        t_[:], xT_d.ap()[k * 128:(k + 1) * 128, 0:TC1])
                    xt0.append(t_)
                    load_wqk(k)
                    t_ = wpool.tile([128, CL], BF16, tag=f"wv{k}", name=f"wv{k}")
                    nc.sync.dma_start(
                        t_[:], wvT_d.ap()[k * 128:(k + 1) * 128, :])
                    wv.append(t_)

                # chunk 0: k-outer over 8 concurrent psum groups, so compute
                # streams behind the interleaved DMA instead of waiting for
                # a full accumulation group's inputs
                grp = [ps1.tile([128, TC1], F32, tag="pqk",
                                name=f"pq0_{m}") for m in range(4)]
                grv = [ps1.tile([128, CL], F32, tag="pv",
                                name=f"pv0_{t}") for t in range(4)]
                for k in range(KT):
                    for m in range(4):
                        nc.tensor.matmul(grp[m][:], wqk[k][m][:], xt0[k][:],
                                         start=(k == 0), stop=(k == KT - 1))
                    for tt in range(4):
                        nc.tensor.matmul(
                            grv[tt][:],
                            xt0[k][:, tt * 128:(tt + 1) * 128],
                            wv[k][:],
                            start=(k == 0), stop=(k == KT - 1))
                for m in range(4):
                    dest = qT[m] if m < HL else kT[m - HL]
                    nc.vector.tensor_copy(dest[:, 0:TC1], grp[m][:])
                    nc.scalar.copy(V[m][:], grv[m][:])

                for j in range(1, NC1):
                    xt = load_x_chunk(j)
                    # pair one qk group with one V group per k-step: the
                    # N=512 and N=256 matmuls hide each other's LDWEIGHTS
                    for m in range(4):
                        pq = ps1.tile([128, TC1], F32, tag="pqk")
                        pv = ps1.tile([128, CL], F32, tag="pv")
                        for k in range(KT):
                            nc.tensor.matmul(pq[:], wqk[k][m][:], xt[k][:],
                                             start=(k == 0), stop=(k == KT - 1))
                            nc.tensor.matmul(
                                pv[:],
                                xt[k][:, m * 128:(m + 1) * 128],
                                wv[k][:],
                                start=(k == 0), stop=(k == KT - 1))
                        dest = qT[m] if m < HL else kT[m - HL]
                        nc.vector.tensor_copy(
                            dest[:, j * TC1:(j + 1) * TC1], pq[:])
                        nc.scalar.copy(V[j * (TC1 // 128) + m][:], pv[:])

            # ---------------- phases 2+3 pools ----------------
            with tc.tile_pool(name="ph2", bufs=4) as p2, \
                 tc.tile_pool(name="es", bufs=2) as es, \
                 tc.tile_pool(name="dn", bufs=2) as dn, \
                 tc.tile_pool(name="a2s", bufs=3) as a2s, \
                 tc.tile_pool(name="p3a", bufs=1) as p3a, \
                 tc.tile_pool(name="p3w", bufs=1) as p3w, \
                 tc.tile_pool(name="acc3", bufs=1) as acc3, \
                 tc.tile_pool(name="p3o", bufs=2) as p3o:
                # prefetch the full projection weight during phase 2:
                # these DMAs sit on the sync queue ahead of the att writes
                wp = []
                for oc in range(C // 512):
                    row = []
                    for kc in range(KT):
                        t_ = p3w.tile([128, 512], BF16, tag=f"wp{oc}_{kc}",
                                      name=f"wp{oc}_{kc}")
                        nc.sync.dma_start(
                            t_[:],
                            wpT_d.ap()[kc * 128:(kc + 1) * 128,
                                       oc * 512:(oc + 1) * 512],
                        )
                        row.append(t_)
                    wp.append(row)

                attn = [None] * KT

                def load_attn(h):
                    for i in range(W):
                        kc = i * HL + h
                        t_ = p3a.tile([128, TL], BF16, tag=f"at{kc}",
                                      name=f"at{kc}")
                        nc.sync.dma_start(t_[:], a2a_out[h][i, :, :])
                        attn[kc] = t_

                # one PSUM pool level across phases 2+3 (8 banks total):
                # paired scores 2x[128,1024] (4 banks) + po 2 + phase-3 2
                with tc.tile_pool(name="ps2s", bufs=2, space="PSUM") as ps2s, \
                     tc.tile_pool(name="ps2o", bufs=2, space="PSUM") as ps2o, \
                     tc.tile_pool(name="ps3", bufs=2, space="PSUM") as ps3:
                    # ------------- phase 2: attention -------------
                    # flat software pipeline over all (head, chunk, pair)
                    # steps: score-pair emission runs a constant 2 pairs
                    # ahead of consumption ACROSS chunk boundaries, so the
                    # tensor engine never drains while the scalar engine
                    # catches up on exp at a chunk start
                    chunks = [(h, j) for h in range(HL) for j in range(NC2)]
                    steps = [(ci, p)
                             for ci, (h, j) in enumerate(chunks)
                             for p in range((j + 1) * 2)]
                    cstate = {}

                    def chunk_state(ci):
                        if ci not in cstate:
                            h, j = chunks[ci]
                            cstate[ci] = {
                                "po": ps2o.tile([128, TC2], F32, tag="po",
                                                name=f"po_{ci}"),
                                "esum": es.tile([128, TC2], FP16, tag="esum",
                                                name=f"esum_{ci}"),
                                "qs": qT[h][:, j * TC2:(j + 1) * TC2],
                            }
                        return cstate[ci]

                    def emit_pair(ci, p):
                        # two score blocks into one 2-bank psum tile
                        h, j = chunks[ci]
                        st = chunk_state(ci)
                        ps = ps2s.tile([128, 2 * TC2], F32, tag="ps")
                        for half in range(2):
                            k = 2 * p + half
                            nc.tensor.matmul(
                                ps[:, half * TC2:(half + 1) * TC2],
                                kT[h][:, k * 128:(k + 1) * 128],
                                st["qs"], start=True, stop=True)
                        return ps

                    LA = 2
                    emitted = {}
                    for i in range(min(LA, len(steps))):
                        emitted[steps[i]] = emit_pair(*steps[i])
                    for i, (ci, p) in enumerate(steps):
                        h, j = chunks[ci]
                        nk = (j + 1) * (TC2 // 128)
                        st = chunk_state(ci)
                        ps = emitted.pop((ci, p))
                        e = p2.tile([128, 2 * TC2], FP16, tag="e")
                        # one exp per pair amortizes ACT init cost
                        nc.scalar.activation(
                            e[:], ps[:],
                            mybir.ActivationFunctionType.Exp,
                            scale=SCALE)
                        for half in range(2):
                            k = 2 * p + half
                            eh = e[:, half * TC2:(half + 1) * TC2]
                            dk = k - 4 * j
                            if dk >= 0:
                                # diagonal: zero out s > t entries
                                nc.vector.tensor_mul(eh, eh, masks[dk][:])
                            if k == 0:
                                nc.vector.tensor_copy(st["esum"][:], eh)
                            else:
                                nc.vector.tensor_add(st["esum"][:],
                                                     st["esum"][:], eh)
                        if i + LA < len(steps):
                            emitted[steps[i + LA]] = emit_pair(*steps[i + LA])
                        for half in range(2):
                            k = 2 * p + half
                            nc.tensor.matmul(
                                st["po"][:],
                                V[k][:, h * 128:(h + 1) * 128],
                                e[:, half * TC2:(half + 1) * TC2],
                                start=(k == 0), stop=(k == nk - 1))
                        if 2 * p + 2 != nk:
                            continue
                        # chunk tail: partition-reduce esum (result arrives
                        # broadcast on all 128 partitions), reciprocal, then
                        # normalize po during PSUM evacuation
                        den = dn.tile([128, TC2], F32, tag="den")
                        nc.gpsimd.partition_all_reduce(
                            den[:], st["esum"][:], channels=128,
                            reduce_op=bass_isa.ReduceOp.add)
                        rec = dn.tile([128, TC2], F32, tag="rec")
                        nc.vector.reciprocal_approx_fast(
                            out=rec[:], in_=den[:])
                        att = a2s.tile([128, TC2], BF16, tag="att")
                        nc.vector.tensor_mul(att[:], st["po"][:], rec[:])
                        nc.sync.dma_start(a2a_in[h][j, :, :], att[:])
                        if j == NC2 - 1:
                            # fire this head's A2A as soon as its chunks are
                            # written; head 0's collective overlaps head 1
                            nc.gpsimd.collective_compute(
                                "AllToAll",
                                mybir.AluOpType.bypass,
                                ins=[a2a_in[h].opt()],
                                outs=[a2a_out[h].opt()],
                                replica_groups=[list(range(W))],
                            )
                            load_attn(h)

                    # ------------- phase 3: output projection -------------
                    # attn arrives normalized. All 16 groups' even-kc halves
                    # (head-0 sourced, available before the second A2A) run
                    # first into rotating psum banks with f32 SBUF spill, so
                    # the tensor engine covers the A2A + odd-tile DMA window;
                    # odd halves then finish in psum and DVE adds the spill.
                    accs = {}
                    for oc in range(4):
                        for tt in range(TL // 128):
                            p3 = ps3.tile([128, 512], F32, tag="p3")
                            for kc in range(0, KT, 2):
                                nc.tensor.matmul(
                                    p3[:],
                                    attn[kc][:, tt * 128:(tt + 1) * 128],
                                    wp[oc][kc][:],
                                    start=(kc == 0), stop=(kc == KT - 2))
                            acc = acc3.tile([128, 512], F32,
                                            tag=f"acc{oc}_{tt}",
                                            name=f"acc{oc}_{tt}")
                            nc.scalar.copy(acc[:], p3[:])
                            accs[(oc, tt)] = acc
                    for oc in range(4):
                        for tt in range(TL // 128):
                            p3 = ps3.tile([128, 512], F32, tag="p3")
                            for kc in range(1, KT, 2):
                                nc.tensor.matmul(
                                    p3[:],
                                    attn[kc][:, tt * 128:(tt + 1) * 128],
                                    wp[oc][kc][:],
                                    start=(kc == 1), stop=(kc == KT - 1))
                            ob = p3o.tile([128, 512], F32, tag="ob")
                            nc.vector.tensor_add(ob[:], accs[(oc, tt)][:],
                                                 p3[:])
                            nc.sync.dma_start(
                                out_d.ap()[tt * 128:(tt + 1) * 128,
                                           oc * 512:(oc + 1) * 512],
                                ob[:])

    nc.compile()
    return nc


def _maybe_install_trace_hook():
    try:
        import antenv
        from trn_agent_boot.trn_boot import _ntff_profile_via_ctypes
        hook = _ntff_profile_via_ctypes("/opt/axon/libaxon_pjrt.so")
        mod = types.ModuleType("antenv.axon_hooks")
        mod.get_axon_ntff_profile_hook = lambda: hook
        mod.set_axon_ntff_profile_hook = lambda h: None
        sys.modules["antenv.axon_hooks"] = mod
        antenv.axon_hooks = mod
        return True
    except Exception:
        return False


def kernel(x, w_attn, w_proj):
    x = np.ascontiguousarray(x, dtype=np.float32)
    w_attn = np.ascontiguousarray(w_attn, dtype=np.float32)
    w_proj = np.ascontiguousarray(w_proj, dtype=np.float32)

    if "nc" not in _cache:
        _cache["nc"] = _build()
    nc = _cache["nc"]

    xT = np.ascontiguousarray(x.T).astype(NP_BF16)
    wpT = np.ascontiguousarray(w_proj.T).astype(NP_BF16)
    in_maps = []
    for c in range(W):
        r0 = CL * c
        wqk = np.concatenate(
            [w_attn[r0:r0 + CL], w_attn[C + r0:C + r0 + CL]], axis=0)
        wqkT = np.ascontiguousarray(wqk.T).astype(NP_BF16)
        wvT = np.ascontiguousarray(
            w_attn[2 * C + r0:2 * C + r0 + CL].T).astype(NP_BF16)
        in_maps.append({"xT": xT, "wqkT": wqkT, "wvT": wvT, "wpT": wpT})

    trace = TRACE and _maybe_install_trace_hook()
    res = run_bass_kernel_spmd(nc, in_maps, list(range(W)), trace=trace)
    LAST_RESULT["exec_time_ns"] = res.exec_time_ns

    return np.concatenate([res.results[c]["out"] for c in range(W)], axis=0)
